# revision 21
# baseline (speedup 1.0000x reference)
"""Adaptive Wavelet Filter Block — Trainium2 Bass kernel (8 NeuronCores).

Sharding: data-parallel over batch B=8 (one batch element per core).
The per-scale quantile threshold needs the global (B,L) energy tensor, so each
core AllGathers the per-row detail energies (tiny: ~15KB/core) and then every
core runs an identical on-device bisection to the exact order statistic v_k.
The mask (energy > thr) equals (energy > v_k) for the reference's linear
interpolation, so only v_k is needed.

Math notes:
 - Haar tree is computed UNSCALED (pure adds/subs); all 2^-s factors are folded
   into the per-row mask multiplier and reconstruction prescales (exact
   power-of-two foldings where it matters for the mask).
 - detail3 is dead code in the reference (never added back); only approx3 and
   detail0..2 matter, so only scales 0..2 need thresholds.
 - 2x linear upsample (align_corners=False) per output phase is
   out_even[m] = 0.25*in[m-1] + 0.75*in[m], out_odd[m] = 0.75*in[m] + 0.25*in[m+1]
   with edge clamping. Reconstruction levels are computed in a "quarter domain"
   (P_l = prefactor * r_l) so each level is exactly two scalar_tensor_tensor
   ops per output plane:  h_b = D_b*msw + J_nb ;  P_b = 3*J_a + h_b,
   with J = prescale(P_prev) on ScalarE. h and the prescale write IN-PLACE over
   the detail/P tiles, so reconstruction allocates no new SBUF besides outputs.
Layout: sequence N on partitions, fully phase-deinterleaved (16 phases of
[256,1024], each 2 tiles of [128,1024]); every Haar/upsample step is
elementwise between planes; only the 2 boundary planes per level need a
partition-shifted copy (SBUF-SBUF DMA).
"""

import sys
from contextlib import ExitStack

import numpy as np

sys.path.insert(0, "/opt/trn_rl_repo")

import concourse.bass as bass
import concourse.mybir as mybir
import concourse.tile as tile
from concourse import bass_isa
from concourse.bass_utils import run_bass_kernel_spmd

B, N, C = 8, 4096, 1024
NCORES = 8
PH = 16        # phases after full deinterleave
HALVES = 2     # partition tiles per plane
F32 = mybir.dt.float32
ALU = mybir.AluOpType
ACTF = mybir.ActivationFunctionType
AX = mybir.AxisListType

# energy column layout: for scale s, plane j, half h -> col
_EBASE = {0: 0, 1: 16, 2: 24}
_NPLANES_D = {0: 8, 1: 4, 2: 2}
ECOLS = 28
NITER_BISECT = 38


def _ecol(s, j, h):
    return _EBASE[s] + 2 * j + h


def _np_sigmoid_f32(x):
    x = np.float64(x)
    return np.float32(1.0 / (1.0 + np.exp(-x)))


def build_nc(host):
    """Build the Bass graph. `host` carries host-precomputed scalars."""
    nc = bass.Bass(
        "TRN2", target_bir_lowering=False, debug=False, num_devices=NCORES
    )
    x_ext = nc.dram_tensor("x", [N, C], F32, kind="ExternalInput")
    tgt_ext = nc.dram_tensor("tgt", [128, 3], F32, kind="ExternalInput")
    out_ext = nc.dram_tensor("out", [N, C], F32, kind="ExternalOutput")
    import os
    dbg = bool(int(os.environ.get("KERNEL_DEBUG", "0")))
    host = dict(host)
    host["dbg"] = dbg
    if dbg:
        host["dbg_tensors"] = {
            "dbgE": nc.dram_tensor("dbgE", [128, ECOLS], F32, kind="ExternalOutput"),
            "dbgEG": nc.dram_tensor("dbgEG", [128, NCORES, ECOLS], F32, kind="ExternalOutput"),
            "dbgvk": nc.dram_tensor("dbgvk", [128, 3], F32, kind="ExternalOutput"),
            "dbgmsw": nc.dram_tensor("dbgmsw", [128, ECOLS], F32, kind="ExternalOutput"),
        }

    xr = x_ext.ap().rearrange("(m p) c -> p m c", p=PH)
    outr = out_ext.ap().rearrange("(m p) c -> p m c", p=PH)

    ctx = ExitStack()
    with tile.TileContext(nc) as tc:
        with ctx:
            _build_body(ctx, tc, nc, xr, outr, tgt_ext, host)
    _split_excess_waits(nc)
    return nc


def _split_excess_waits(nc):
    """This walrus build accepts at most ONE embedded sem-wait per
    instruction; hoist surplus waits into standalone EventSemaphore
    instructions on the same engine, placed immediately before."""
    import copy
    import bass_rust
    # template: any well-formed EventSemaphore (the tail drain barrier has one
    # per engine)
    tmpl = None
    for f in nc.m.functions:
        for b in f.blocks:
            for inst in b.instructions:
                if type(inst).__name__ == "InstEventSemaphore":
                    tmpl = inst
    assert tmpl is not None, "no EventSemaphore template found"
    n_split = 0
    for f in nc.m.functions:
        for b in f.blocks:
            new_insts = []
            for inst in b.instructions:
                si = inst.sync_info
                if si is not None and len(si.on_wait) > 1:
                    waits = list(si.on_wait)
                    for k, w in enumerate(waits[:-1]):
                        nop = copy.deepcopy(tmpl)
                        nop.name = f"{inst.name}-w{k}"
                        nop.engine = inst.engine
                        nop.sync_info = bass_rust.SyncInfo(
                            on_wait=[w], on_update=[])
                        new_insts.append(nop)
                        n_split += 1
                    si.on_wait = [waits[-1]]
                    inst.sync_info = si
                new_insts.append(inst)
            b.instructions[:] = new_insts
    return n_split


def _build_body(ctx, tc, nc, xr, outr, tgt_ext, host):
    pool = ctx.enter_context(tc.tile_pool(name="main", bufs=1))
    psum = ctx.enter_context(tc.tile_pool(name="ps", bufs=1, space="PSUM"))
    dram = ctx.enter_context(tc.tile_pool(name="dram", bufs=1, space="DRAM"))

    v = nc.vector
    gp = nc.gpsimd
    sc = nc.scalar
    te = nc.tensor
    sy = nc.sync

    def big(tag, bufs, name):
        return pool.tile([128, C], F32, tag=tag, bufs=bufs, name=name)

    def small(shape, name):
        return pool.tile(shape, F32, tag=name, bufs=1, name=name)

    E = small([128, ECOLS], "E")            # local energies
    EG = small([128, NCORES, ECOLS], "EG")  # gathered energies
    tgt = small([128, 3], "tgt")
    lo = small([128, 3], "lo")
    hi = small([128, 3], "hi")
    mid = small([128, 3], "mid")
    tmp3 = small([128, 3], "tmp3")
    selg = pool.tile([128, 3], mybir.dt.int32, tag="selg", bufs=1, name="selg")
    sell = pool.tile([128, 3], mybir.dt.int32, tag="sell", bufs=1, name="sell")
    partials = small([128, 3], "partials")
    rmax = small([128, 3], "rmax")
    ones128 = small([128, 128], "ones128")
    msw = small([128, ECOLS], "msw")
    cmp_scr = small([128, NCORES, 16], "cmp_scr")

    sy.dma_start(out=tgt[:, :], in_=tgt_ext.ap())
    v.memset(ones128[:, :], 1.0)
    v.memset(lo[:, :], -1.0)

    # ---- phase 1: load + Haar tree + energies ----
    def new_plane(tag, bufs, name):
        return [big(f"{tag}_h{h}", bufs, f"{name}_h{h}") for h in range(2)]

    # detail/approx plane tiles (persistent; recon reuses them in place)
    D0 = [new_plane("tD0", 8, f"D0_{j}") for j in range(8)]
    D1 = [new_plane("tD1", 4, f"D1_{j}") for j in range(4)]
    D2 = [new_plane("tD2", 2, f"D2_{j}") for j in range(2)]

    # load x phases and compute scale 0; A0/A1/A2 rotate
    A0 = [new_plane("tA0", 2, f"A0_{j}") for j in range(8)]
    xpair = xr.tensor.ap().rearrange("(m pj e) c -> pj m e c", pj=8, e=2)
    for j in range(8):
        for h in range(HALVES):
            xl = pool.tile([128, 2, C], F32, tag="xl", bufs=2, name="xl")
            v.memset(xl[:, 0:1, 0:1], 0.0)
            sy.dma_start(out=xl[:, :, :], in_=xpair[j, 128 * h:128 * (h + 1), :, :])
            gp.tensor_tensor(D0[j][h][:, :], xl[:, 0, :], xl[:, 1, :], ALU.subtract)
            gp.tensor_tensor(A0[j][h][:, :], xl[:, 0, :], xl[:, 1, :], ALU.add)

    def halfop(eng, out_pl, a_pl, b_pl, op):
        for h in range(HALVES):
            eng.tensor_tensor(out_pl[h][:, :], a_pl[h][:, :], b_pl[h][:, :], op)

    A1 = [new_plane("tA1", 2, f"A1_{j}") for j in range(4)]
    A2 = [new_plane("tA2", 2, f"A2_{j}") for j in range(2)]
    A3 = A2[0]
    for j in range(4):
        halfop(gp, A1[j], A0[2 * j], A0[2 * j + 1], ALU.add)
        halfop(gp, D1[j], A0[2 * j], A0[2 * j + 1], ALU.subtract)
    for j in range(2):
        halfop(gp, A2[j], A1[2 * j], A1[2 * j + 1], ALU.add)
        halfop(gp, D2[j], A1[2 * j], A1[2 * j + 1], ALU.subtract)
    halfop(gp, A3, A2[0], A2[1], ALU.add)

    # energies (unscaled): E[:, col] = sum_c D^2 ; Square+accum on ScalarE
    for s, D in ((0, D0), (1, D1), (2, D2)):
        for j in range(_NPLANES_D[s]):
            for h in range(HALVES):
                sq = big("tA0_h0", 2, "sq")
                sc.activation(
                    sq[:, :], D[j][h][:, :], ACTF.Square,
                    accum_out=E[:, _ecol(s, j, h):_ecol(s, j, h) + 1],
                )

    # ---- phase 2: AllGather energies ----
    cc_in = dram.tile([128, ECOLS], F32, tag="cc_in", name="cc_in")
    cc_out = dram.tile([NCORES, 128, ECOLS], F32, tag="cc_out", name="cc_out")
    sy.dma_start(out=cc_in[:, :], in_=E[:, :])
    gp.collective_compute(
        "AllGather", ALU.bypass,
        replica_groups=[list(range(NCORES))],
        ins=[cc_in[:, :].opt()],
        outs=[cc_out[:, :, :].opt()],
    )
    sy.dma_start(
        out=EG[:, :, :],
        in_=cc_out[:, :, :].rearrange("c p j -> p c j"),
    )

    escale = [EG[:, :, 0:16], EG[:, :, 16:24], EG[:, :, 24:28]]

    # ---- phase 3: bisection for v_k per scale ----
    # hi0 = sum over partitions of per-partition maxima (>= global max; costs
    # ~7 extra bisection iterations vs the exact max)
    for s in range(3):
        v.tensor_reduce(rmax[:, s:s + 1], escale[s], AX.XY, ALU.max)
    gmax_ps = psum.tile([128, 3], F32, tag="gmax_ps", name="gmax_ps")
    te.matmul(gmax_ps[:, :], ones128[:, :], rmax[:, :], start=True, stop=True)
    v.tensor_scalar(hi[:, :], gmax_ps[:, :], 1.0001, 1.0, ALU.mult, ALU.add)
    v.tensor_tensor(tmp3[:, :], lo[:, :], hi[:, :], ALU.add)
    v.tensor_scalar_mul(mid[:, :], tmp3[:, :], 0.5)

    cntb_ps = psum.tile([128, 3], F32, tag="cntb_ps", name="cntb_ps")
    for _ in range(NITER_BISECT):
        for s in range(3):
            ncols = [16, 8, 4][s]
            v.tensor_scalar(
                cmp_scr[:, :, :ncols], escale[s], mid[:, s:s + 1], None,
                ALU.is_le, ALU.add, accum_out=partials[:, s:s + 1],
            )
        te.matmul(cntb_ps[:, :], ones128[:, :], partials[:, :],
                  start=True, stop=True)
        v.tensor_tensor(selg[:, :], cntb_ps[:, :], tgt[:, :], ALU.is_ge)
        v.tensor_tensor(sell[:, :], cntb_ps[:, :], tgt[:, :], ALU.is_lt)
        v.copy_predicated(hi[:, :], selg[:, :], mid[:, :])
        v.copy_predicated(lo[:, :], sell[:, :], mid[:, :])
        v.tensor_tensor(tmp3[:, :], lo[:, :], hi[:, :], ALU.add)
        v.tensor_scalar_mul(mid[:, :], tmp3[:, :], 0.5)

    # ---- phase 4: per-row mask multipliers ----
    # msw[:,col] = (E[:,col] > v_k[s]) * csw_s  (csw has recon scales folded)
    for s in range(3):
        c0, c1 = _EBASE[s], _EBASE[s] + 2 * _NPLANES_D[s]
        v.tensor_scalar(
            msw[:, c0:c1], E[:, c0:c1], hi[:, s:s + 1], float(host["csw"][s]),
            ALU.is_gt, ALU.mult,
        )

    if host.get("dbg"):
        dt = host["dbg_tensors"]
        sy.dma_start(out=dt["dbgE"].ap(), in_=E[:, :])
        sy.dma_start(out=dt["dbgEG"].ap(), in_=EG[:, :, :])
        sy.dma_start(out=dt["dbgvk"].ap(), in_=hi[:, :])
        sy.dma_start(out=dt["dbgmsw"].ap(), in_=msw[:, :])

    # ---- phase 5: reconstruction (quarter/P domain, in-place) ----
    def shifted_minus(planes):
        # plane = in[m-1]; in[-1] clamps to global row 0
        src, first = planes[-1], planes[0]
        s_t = [big("shf", 3, "shm") for _ in range(2)]
        for t in s_t:
            v.memset(t[:, 0:1], 0.0)
        sy.dma_start(out=s_t[0][0:1, :], in_=first[0][0:1, :])
        sy.dma_start(out=s_t[0][1:128, :], in_=src[0][0:127, :])
        sy.dma_start(out=s_t[1][0:1, :], in_=src[0][127:128, :])
        sy.dma_start(out=s_t[1][1:128, :], in_=src[1][0:127, :])
        return s_t

    def shifted_plus(planes):
        # plane = in[m+1]; in[M] clamps to global last row
        src, last = planes[0], planes[-1]
        s_t = [big("shf", 3, "shp") for _ in range(2)]
        for t in s_t:
            v.memset(t[:, 0:1], 0.0)
        sy.dma_start(out=s_t[0][0:127, :], in_=src[0][1:128, :])
        sy.dma_start(out=s_t[0][127:128, :], in_=src[1][0:1, :])
        sy.dma_start(out=s_t[1][0:127, :], in_=src[1][1:128, :])
        sy.dma_start(out=s_t[1][127:128, :], in_=last[1][127:128, :])
        return s_t

    def level(J, Dpl, s, up_eng, h_eng, out_planes=None, v_planes=0):
        """J: prescaled input planes. Produces P planes in-place over Dpl
        (P_b = 3*J_a + D_b*msw + J_nb), or into out_planes if given."""
        P = len(J)
        shm = shifted_minus(J)
        shpl = shifted_plus(J)
        outs = []
        for b in range(2 * P):
            a, odd = divmod(b, 2)
            nb = (shm if (not odd and a == 0) else
                  shpl if (odd and a == P - 1) else
                  J[a - 1] if not odd else J[a + 1])
            o = out_planes[b] if out_planes is not None else Dpl[b]
            ue = v if b < v_planes * 2 else up_eng
            for h in range(2):
                if Dpl is not None:
                    col = _ecol(s, b, h)
                    h_eng.scalar_tensor_tensor(
                        Dpl[b][h][:, :], Dpl[b][h][:, :], msw[:, col:col + 1],
                        nb[h][:, :], ALU.mult, ALU.add,
                    )
                    h_in = Dpl[b][h][:, :]
                else:
                    h_in = nb[h][:, :]
                ue.scalar_tensor_tensor(
                    o[h][:, :], J[a][h][:, :], 3.0, h_in, ALU.mult, ALU.add
                )
            outs.append(o)
        return outs

    def prescale_inplace(planes, scale):
        for pl in planes:
            for h in range(2):
                sc.activation(pl[h][:, :], pl[h][:, :], ACTF.Copy, 0.0,
                              float(scale))
        return planes

    # J3 = A3' * sw3/256  (= r3/16)
    J3 = prescale_inplace([A3], host["j3_scale"])
    P2 = level(J3, D2, 2, v, v)                  # P2 = r2/4   (in D2 tiles)
    J2 = prescale_inplace(P2, 0.25)               # J2 = r2/16
    P1 = level(J2, D1, 1, v, v)                  # P1 = r1/4   (in D1 tiles)
    J1 = prescale_inplace(P1, host["j1_scale"])   # J1 = g*r1/16
    P0 = level(J1, D0, 0, v, v)                  # P0 = g*r0/4 (in D0 tiles)
    outpl = [new_plane("outp", 2, f"o_{b}") for b in range(PH)]
    outpl = level(P0, None, None, v, v, out_planes=outpl)

    # ---- phase 6: store ----
    for p in range(PH):
        for h in range(HALVES):
            sy.dma_start(
                out=outr[p, 128 * h:128 * (h + 1), :],
                in_=outpl[p][h][:, :],
            )


def _host_scalars(cw1, cw2, cw3, threshold_param, scale_weights,
                  reconstruction_weight):
    f32 = np.float32
    sw = [_np_sigmoid_f32(scale_weights[i]) for i in range(4)]
    w1 = f32(np.mean(f32(cw1)))
    w2 = f32(np.mean(f32(cw2)))
    w3 = f32(np.mean(f32(cw3)))
    mix = f32(f32(w1 + f32(f32(0.5) * w2)) + f32(f32(0.2) * w3))
    g = f32(mix * _np_sigmoid_f32(reconstruction_weight[0]))
    q = f32(threshold_param[0])

    targets = []
    for s in range(3):
        n_s = B * (N >> (s + 1))
        idx = f32(q * f32(n_s - 1))
        k = int(np.clip(np.floor(idx), 0, n_s - 1))
        targets.append(float(k + 1))

    # mask multipliers with recon quarter-domain factors folded in:
    # scale0: 0.25*g*(sw0/2); scale1: 0.25*(sw1/4); scale2: 0.25*(sw2/8)
    csw = [
        f32(f32(f32(sw[0] / f32(2.0)) * f32(0.25)) * g),
        f32(f32(sw[1] / f32(4.0)) * f32(0.25)),
        f32(f32(sw[2] / f32(8.0)) * f32(0.25)),
    ]
    return {
        "g": g, "csw": csw,
        "j3_scale": f32(sw[3] / f32(256.0)),
        "j1_scale": f32(g * f32(0.25)),
        "targets": np.tile(np.array(targets, np.float32), (128, 1)),
    }


def _install_ntff_hook_shim(so_path="/opt/axon/libaxon_pjrt.so"):
    """Provide antenv.axon_hooks (missing in this image) so
    run_bass_kernel_spmd(trace=True) can capture NTFF profiles."""
    import sys as _sys
    import types, ctypes, contextlib
    if "antenv.axon_hooks" in _sys.modules:
        return
    lib = ctypes.CDLL(so_path)
    if not hasattr(lib, "axon_start_nrt_profile"):
        return
    lib.axon_start_nrt_profile.argtypes = [
        ctypes.POINTER(ctypes.c_int64), ctypes.c_size_t]
    lib.axon_start_nrt_profile.restype = ctypes.c_int64
    lib.axon_stop_nrt_profile.argtypes = [ctypes.c_char_p]
    lib.axon_stop_nrt_profile.restype = ctypes.c_int64

    @contextlib.contextmanager
    def _hook(output_dir, device_ids):
        import jax
        jax.devices()
        if device_ids:
            ids = (ctypes.c_int64 * len(device_ids))(*device_ids)
            rc = lib.axon_start_nrt_profile(ids, len(device_ids))
        else:
            rc = lib.axon_start_nrt_profile(None, 0)
        if rc != 0:
            raise RuntimeError(f"axon_start_nrt_profile rc={rc}")
        try:
            yield
        finally:
            n = lib.axon_stop_nrt_profile(str(output_dir).encode())
            print(f"ntff profile: {n} file(s) -> {output_dir}")

    mod = types.ModuleType("antenv.axon_hooks")
    mod.get_axon_ntff_profile_hook = lambda: _hook
    mod.set_axon_ntff_profile_hook = lambda h: None
    import antenv
    antenv.axon_hooks = mod
    _sys.modules["antenv.axon_hooks"] = mod


def kernel(x, cw1, cw2, cw3, threshold_param, scale_weights,
           reconstruction_weight):
    host = _host_scalars(cw1, cw2, cw3, threshold_param, scale_weights,
                         reconstruction_weight)
    nc = build_nc(host)
    in_maps = [
        {"x": np.ascontiguousarray(x[c]), "tgt": host["targets"]}
        for c in range(NCORES)
    ]
    import os
    trace = bool(int(os.environ.get("KERNEL_TRACE", "0")))
    if trace:
        _install_ntff_hook_shim()
    res = run_bass_kernel_spmd(
        nc, in_maps, core_ids=list(range(NCORES)), trace=trace
    )
    if trace:
        print("HW exec time:", res.exec_time_ns, "ns")
        kernel.last_result = res
    out = np.stack([res.results[c]["out"] for c in range(NCORES)], axis=0)
    return out.astype(np.float32)


if __name__ == "__main__":
    host = _host_scalars(
        np.zeros((1, 1, C), np.float32), np.zeros((1, 1, C), np.float32),
        np.zeros((1, 1, C), np.float32), np.array([0.25], np.float32),
        np.zeros((4,), np.float32), np.zeros((1,), np.float32),
    )
    nc = build_nc(host)
    print("built ok")


# revision 23
# speedup vs baseline: 1.0412x; 1.0412x over previous
"""Adaptive Wavelet Filter Block — Trainium2 Bass kernel (8 NeuronCores).

Sharding: data-parallel over batch B=8 (one batch element per core).
The per-scale quantile threshold needs the global (B,L) energy tensor, so each
core AllGathers the per-row detail energies (tiny: ~15KB/core) and then every
core runs an identical on-device bisection to the exact order statistic v_k.
The mask (energy > thr) equals (energy > v_k) for the reference's linear
interpolation, so only v_k is needed.

Math notes:
 - Haar tree is computed UNSCALED (pure adds/subs); all 2^-s factors are folded
   into the per-row mask multiplier and reconstruction prescales (exact
   power-of-two foldings where it matters for the mask).
 - detail3 is dead code in the reference (never added back); only approx3 and
   detail0..2 matter, so only scales 0..2 need thresholds.
 - 2x linear upsample (align_corners=False) per output phase is
   out_even[m] = 0.25*in[m-1] + 0.75*in[m], out_odd[m] = 0.75*in[m] + 0.25*in[m+1]
   with edge clamping. Reconstruction levels are computed in a "quarter domain"
   (P_l = prefactor * r_l) so each level is exactly two scalar_tensor_tensor
   ops per output plane:  h_b = D_b*msw + J_nb ;  P_b = 3*J_a + h_b,
   with J = prescale(P_prev) on ScalarE. h and the prescale write IN-PLACE over
   the detail/P tiles, so reconstruction allocates no new SBUF besides outputs.
Layout: sequence N on partitions, fully phase-deinterleaved (16 phases of
[256,1024], each 2 tiles of [128,1024]); every Haar/upsample step is
elementwise between planes; only the 2 boundary planes per level need a
partition-shifted copy (SBUF-SBUF DMA).
"""

import sys
from contextlib import ExitStack

import numpy as np

sys.path.insert(0, "/opt/trn_rl_repo")

import concourse.bass as bass
import concourse.mybir as mybir
import concourse.tile as tile
from concourse import bass_isa
from concourse.bass_utils import run_bass_kernel_spmd

B, N, C = 8, 4096, 1024
NCORES = 8
PH = 16        # phases after full deinterleave
HALVES = 2     # partition tiles per plane
F32 = mybir.dt.float32
ALU = mybir.AluOpType
ACTF = mybir.ActivationFunctionType
AX = mybir.AxisListType

# energy column layout: for scale s, plane j, half h -> col
_EBASE = {0: 0, 1: 16, 2: 24}
_NPLANES_D = {0: 8, 1: 4, 2: 2}
ECOLS = 28
NITER_BISECT = 38


def _ecol(s, j, h):
    return _EBASE[s] + 2 * j + h


def _np_sigmoid_f32(x):
    x = np.float64(x)
    return np.float32(1.0 / (1.0 + np.exp(-x)))


def build_nc(host):
    """Build the Bass graph. `host` carries host-precomputed scalars."""
    nc = bass.Bass(
        "TRN2", target_bir_lowering=False, debug=False, num_devices=NCORES
    )
    x_ext = nc.dram_tensor("x", [N, C], F32, kind="ExternalInput")
    tgt_ext = nc.dram_tensor("tgt", [128, 3], F32, kind="ExternalInput")
    out_ext = nc.dram_tensor("out", [N, C], F32, kind="ExternalOutput")
    import os
    dbg = bool(int(os.environ.get("KERNEL_DEBUG", "0")))
    host = dict(host)
    host["dbg"] = dbg
    if dbg:
        host["dbg_tensors"] = {
            "dbgE": nc.dram_tensor("dbgE", [128, ECOLS], F32, kind="ExternalOutput"),
            "dbgEG": nc.dram_tensor("dbgEG", [128, NCORES, ECOLS], F32, kind="ExternalOutput"),
            "dbgvk": nc.dram_tensor("dbgvk", [128, 3], F32, kind="ExternalOutput"),
            "dbgmsw": nc.dram_tensor("dbgmsw", [128, ECOLS], F32, kind="ExternalOutput"),
        }

    xr = x_ext.ap().rearrange("(m p) c -> p m c", p=PH)
    outr = out_ext.ap().rearrange("(m p) c -> p m c", p=PH)

    ctx = ExitStack()
    with tile.TileContext(nc) as tc:
        with ctx:
            _build_body(ctx, tc, nc, xr, outr, tgt_ext, host)
    _split_excess_waits(nc)
    return nc


def _split_excess_waits(nc):
    """This walrus build accepts at most ONE embedded sem-wait per
    instruction; hoist surplus waits into standalone EventSemaphore
    instructions on the same engine, placed immediately before."""
    import copy
    import bass_rust
    # template: any well-formed EventSemaphore (the tail drain barrier has one
    # per engine)
    tmpl = None
    for f in nc.m.functions:
        for b in f.blocks:
            for inst in b.instructions:
                if type(inst).__name__ == "InstEventSemaphore":
                    tmpl = inst
    assert tmpl is not None, "no EventSemaphore template found"
    n_split = 0
    for f in nc.m.functions:
        for b in f.blocks:
            new_insts = []
            for inst in b.instructions:
                si = inst.sync_info
                if si is not None and len(si.on_wait) > 1:
                    waits = list(si.on_wait)
                    for k, w in enumerate(waits[:-1]):
                        nop = copy.deepcopy(tmpl)
                        nop.name = f"{inst.name}-w{k}"
                        nop.engine = inst.engine
                        nop.sync_info = bass_rust.SyncInfo(
                            on_wait=[w], on_update=[])
                        new_insts.append(nop)
                        n_split += 1
                    si.on_wait = [waits[-1]]
                    inst.sync_info = si
                new_insts.append(inst)
            b.instructions[:] = new_insts
    return n_split


def _build_body(ctx, tc, nc, xr, outr, tgt_ext, host):
    pool = ctx.enter_context(tc.tile_pool(name="main", bufs=1))
    psum = ctx.enter_context(tc.tile_pool(name="ps", bufs=1, space="PSUM"))
    dram = ctx.enter_context(tc.tile_pool(name="dram", bufs=1, space="DRAM"))

    v = nc.vector
    gp = nc.gpsimd
    sc = nc.scalar
    te = nc.tensor
    sy = nc.sync

    def big(tag, bufs, name):
        return pool.tile([128, C], F32, tag=tag, bufs=bufs, name=name)

    def small(shape, name):
        return pool.tile(shape, F32, tag=name, bufs=1, name=name)

    E = small([128, ECOLS], "E")            # local energies
    EG = small([128, NCORES, ECOLS], "EG")  # gathered energies
    tgt = small([128, 3], "tgt")
    lo = small([128, 3], "lo")
    hi = small([128, 3], "hi")
    mid = small([128, 3], "mid")
    tmp3 = small([128, 3], "tmp3")
    selg = pool.tile([128, 3], mybir.dt.int32, tag="selg", bufs=1, name="selg")
    sell = pool.tile([128, 3], mybir.dt.int32, tag="sell", bufs=1, name="sell")
    partials = small([128, 3], "partials")
    rmax = small([128, 3], "rmax")
    ones128 = small([128, 128], "ones128")
    msw = small([128, ECOLS], "msw")
    cmp_scr = small([128, NCORES, 16], "cmp_scr")

    sy.dma_start(out=tgt[:, :], in_=tgt_ext.ap())
    v.memset(ones128[:, :], 1.0)
    v.memset(lo[:, :], -1.0)

    # ---- phase 1: load + Haar tree + energies ----
    def new_plane(tag, bufs, name):
        return [big(f"{tag}_h{h}", bufs, f"{name}_h{h}") for h in range(2)]

    # detail/approx plane tiles (persistent; recon reuses them in place)
    D0 = [new_plane("tD0", 8, f"D0_{j}") for j in range(8)]
    D1 = [new_plane("tD1", 4, f"D1_{j}") for j in range(4)]
    D2 = [new_plane("tD2", 2, f"D2_{j}") for j in range(2)]

    # load x phases and compute scale 0; A0/A1/A2 rotate
    A0 = [new_plane("tA0", 2, f"A0_{j}") for j in range(8)]
    xpair = xr.tensor.ap().rearrange("(m pj e) c -> pj m e c", pj=8, e=2)
    for j in range(8):
        for h in range(HALVES):
            xl = pool.tile([128, 2, C], F32, tag="xl", bufs=2, name="xl")
            v.memset(xl[:, 0:1, 0:1], 0.0)
            (sy if (j + h) % 2 == 0 else sc).dma_start(out=xl[:, :, :], in_=xpair[j, 128 * h:128 * (h + 1), :, :])
            v.tensor_tensor(D0[j][h][:, :], xl[:, 0, :], xl[:, 1, :], ALU.subtract)
            gp.tensor_tensor(A0[j][h][:, :], xl[:, 0, :], xl[:, 1, :], ALU.add)

    def halfop(eng, out_pl, a_pl, b_pl, op):
        for h in range(HALVES):
            eng.tensor_tensor(out_pl[h][:, :], a_pl[h][:, :], b_pl[h][:, :], op)

    A1 = [new_plane("tA1", 2, f"A1_{j}") for j in range(4)]
    A2 = [new_plane("tA2", 2, f"A2_{j}") for j in range(2)]
    A3 = A2[0]
    for j in range(4):
        halfop(gp, A1[j], A0[2 * j], A0[2 * j + 1], ALU.add)
        halfop(v, D1[j], A0[2 * j], A0[2 * j + 1], ALU.subtract)
    for j in range(2):
        halfop(gp, A2[j], A1[2 * j], A1[2 * j + 1], ALU.add)
        halfop(v, D2[j], A1[2 * j], A1[2 * j + 1], ALU.subtract)
    halfop(gp, A3, A2[0], A2[1], ALU.add)

    # energies (unscaled): E[:, col] = sum_c D^2 ; Square+accum on ScalarE
    for s, D in ((0, D0), (1, D1), (2, D2)):
        for j in range(_NPLANES_D[s]):
            for h in range(HALVES):
                sq = big("shf", 3, "sq")
                sc.activation(
                    sq[:, :], D[j][h][:, :], ACTF.Square,
                    accum_out=E[:, _ecol(s, j, h):_ecol(s, j, h) + 1],
                )

    # ---- phase 2: AllGather energies ----
    cc_in = dram.tile([128, ECOLS], F32, tag="cc_in", name="cc_in")
    cc_out = dram.tile([NCORES, 128, ECOLS], F32, tag="cc_out", name="cc_out")
    sy.dma_start(out=cc_in[:, :], in_=E[:, :])
    gp.collective_compute(
        "AllGather", ALU.bypass,
        replica_groups=[list(range(NCORES))],
        ins=[cc_in[:, :].opt()],
        outs=[cc_out[:, :, :].opt()],
    )
    for cc_i in range(NCORES):
        egst = pool.tile([128, ECOLS], F32, tag="egst", bufs=4, name="egst")
        (sy if cc_i % 2 == 0 else sc).dma_start(
            out=egst[:, :], in_=cc_out[cc_i, :, :])
        v.tensor_copy(EG[:, cc_i, :], egst[:, :])

    escale = [EG[:, :, 0:16], EG[:, :, 16:24], EG[:, :, 24:28]]

    # ---- phase 3: bisection for v_k per scale ----
    # hi0 = sum over partitions of per-partition maxima (>= global max; costs
    # ~7 extra bisection iterations vs the exact max)
    for s in range(3):
        v.tensor_reduce(rmax[:, s:s + 1], escale[s], AX.XY, ALU.max)
    gmax_ps = psum.tile([128, 3], F32, tag="gmax_ps", name="gmax_ps")
    te.matmul(gmax_ps[:, :], ones128[:, :], rmax[:, :], start=True, stop=True)
    v.tensor_scalar(hi[:, :], gmax_ps[:, :], 1.0001, 1.0, ALU.mult, ALU.add)
    v.tensor_tensor(tmp3[:, :], lo[:, :], hi[:, :], ALU.add)
    v.tensor_scalar_mul(mid[:, :], tmp3[:, :], 0.5)

    cntb_ps = psum.tile([128, 3], F32, tag="cntb_ps", name="cntb_ps")
    for _ in range(NITER_BISECT):
        for s in range(3):
            ncols = [16, 8, 4][s]
            v.tensor_scalar(
                cmp_scr[:, :, :ncols], escale[s], mid[:, s:s + 1], None,
                ALU.is_le, ALU.add, accum_out=partials[:, s:s + 1],
            )
        te.matmul(cntb_ps[:, :], ones128[:, :], partials[:, :],
                  start=True, stop=True)
        v.tensor_tensor(selg[:, :], cntb_ps[:, :], tgt[:, :], ALU.is_ge)
        v.tensor_tensor(sell[:, :], cntb_ps[:, :], tgt[:, :], ALU.is_lt)
        v.copy_predicated(hi[:, :], selg[:, :], mid[:, :])
        v.copy_predicated(lo[:, :], sell[:, :], mid[:, :])
        v.tensor_tensor(tmp3[:, :], lo[:, :], hi[:, :], ALU.add)
        v.tensor_scalar_mul(mid[:, :], tmp3[:, :], 0.5)

    # ---- phase 4: per-row mask multipliers ----
    # msw[:,col] = (E[:,col] > v_k[s]) * csw_s  (csw has recon scales folded)
    for s in range(3):
        c0, c1 = _EBASE[s], _EBASE[s] + 2 * _NPLANES_D[s]
        v.tensor_scalar(
            msw[:, c0:c1], E[:, c0:c1], hi[:, s:s + 1], float(host["csw"][s]),
            ALU.is_gt, ALU.mult,
        )

    if host.get("dbg"):
        dt = host["dbg_tensors"]
        sy.dma_start(out=dt["dbgE"].ap(), in_=E[:, :])
        sy.dma_start(out=dt["dbgEG"].ap(), in_=EG[:, :, :])
        sy.dma_start(out=dt["dbgvk"].ap(), in_=hi[:, :])
        sy.dma_start(out=dt["dbgmsw"].ap(), in_=msw[:, :])

    # ---- phase 5: reconstruction (quarter/P domain, in-place) ----
    def shifted_minus(planes):
        # plane = in[m-1]; in[-1] clamps to global row 0
        src, first = planes[-1], planes[0]
        s_t = [big("shf", 3, "shm") for _ in range(2)]
        for t in s_t:
            v.memset(t[:, 0:1], 0.0)
        sy.dma_start(out=s_t[0][0:1, :], in_=first[0][0:1, :])
        sy.dma_start(out=s_t[0][1:128, :], in_=src[0][0:127, :])
        sy.dma_start(out=s_t[1][0:1, :], in_=src[0][127:128, :])
        sy.dma_start(out=s_t[1][1:128, :], in_=src[1][0:127, :])
        return s_t

    def shifted_plus(planes):
        # plane = in[m+1]; in[M] clamps to global last row
        src, last = planes[0], planes[-1]
        s_t = [big("shf", 3, "shp") for _ in range(2)]
        for t in s_t:
            v.memset(t[:, 0:1], 0.0)
        sy.dma_start(out=s_t[0][0:127, :], in_=src[0][1:128, :])
        sy.dma_start(out=s_t[0][127:128, :], in_=src[1][0:1, :])
        sy.dma_start(out=s_t[1][0:127, :], in_=src[1][1:128, :])
        sy.dma_start(out=s_t[1][127:128, :], in_=last[1][127:128, :])
        return s_t

    def level(J, Dpl, s, up_eng, h_eng, out_planes=None, v_planes=0):
        """J: prescaled input planes. Produces P planes in-place over Dpl
        (P_b = 3*J_a + D_b*msw + J_nb), or into out_planes if given."""
        P = len(J)
        shm = shifted_minus(J)
        shpl = shifted_plus(J)
        outs = []
        for b in range(2 * P):
            a, odd = divmod(b, 2)
            nb = (shm if (not odd and a == 0) else
                  shpl if (odd and a == P - 1) else
                  J[a - 1] if not odd else J[a + 1])
            o = out_planes[b] if out_planes is not None else Dpl[b]
            ue = v if b < v_planes * 2 else up_eng
            for h in range(2):
                if Dpl is not None:
                    col = _ecol(s, b, h)
                    sc.activation(Dpl[b][h][:, :], Dpl[b][h][:, :], ACTF.Copy,
                                  0.0, msw[:, col:col + 1])
                    gp.tensor_tensor(Dpl[b][h][:, :], Dpl[b][h][:, :],
                                     nb[h][:, :], ALU.add)
                    h_in = Dpl[b][h][:, :]
                else:
                    h_in = nb[h][:, :]
                ue.scalar_tensor_tensor(
                    o[h][:, :], J[a][h][:, :], 3.0, h_in, ALU.mult, ALU.add
                )
            outs.append(o)
        return outs

    def prescale_inplace(planes, scale):
        for pl in planes:
            for h in range(2):
                sc.activation(pl[h][:, :], pl[h][:, :], ACTF.Copy, 0.0,
                              float(scale))
        return planes

    # J3 = A3' * sw3/256  (= r3/16)
    J3 = prescale_inplace([A3], host["j3_scale"])
    P2 = level(J3, D2, 2, v, v)                  # P2 = r2/4   (in D2 tiles)
    J2 = prescale_inplace(P2, 0.25)               # J2 = r2/16
    P1 = level(J2, D1, 1, v, v)                  # P1 = r1/4   (in D1 tiles)
    J1 = prescale_inplace(P1, host["j1_scale"])   # J1 = g*r1/16
    P0 = level(J1, D0, 0, v, v)                  # P0 = g*r0/4 (in D0 tiles)
    outpl = [new_plane("outp", 2, f"o_{b}") for b in range(PH)]
    outpl = level(P0, None, None, v, v, out_planes=outpl)

    # ---- phase 6: store ----
    for p in range(PH):
        for h in range(HALVES):
            eng = (sy, sc, gp)[(p * 2 + h) % 3]
            eng.dma_start(
                out=outr[p, 128 * h:128 * (h + 1), :],
                in_=outpl[p][h][:, :],
            )


def _host_scalars(cw1, cw2, cw3, threshold_param, scale_weights,
                  reconstruction_weight):
    f32 = np.float32
    sw = [_np_sigmoid_f32(scale_weights[i]) for i in range(4)]
    w1 = f32(np.mean(f32(cw1)))
    w2 = f32(np.mean(f32(cw2)))
    w3 = f32(np.mean(f32(cw3)))
    mix = f32(f32(w1 + f32(f32(0.5) * w2)) + f32(f32(0.2) * w3))
    g = f32(mix * _np_sigmoid_f32(reconstruction_weight[0]))
    q = f32(threshold_param[0])

    targets = []
    for s in range(3):
        n_s = B * (N >> (s + 1))
        idx = f32(q * f32(n_s - 1))
        k = int(np.clip(np.floor(idx), 0, n_s - 1))
        targets.append(float(k + 1))

    # mask multipliers with recon quarter-domain factors folded in:
    # scale0: 0.25*g*(sw0/2); scale1: 0.25*(sw1/4); scale2: 0.25*(sw2/8)
    csw = [
        f32(f32(f32(sw[0] / f32(2.0)) * f32(0.25)) * g),
        f32(f32(sw[1] / f32(4.0)) * f32(0.25)),
        f32(f32(sw[2] / f32(8.0)) * f32(0.25)),
    ]
    return {
        "g": g, "csw": csw,
        "j3_scale": f32(sw[3] / f32(256.0)),
        "j1_scale": f32(g * f32(0.25)),
        "targets": np.tile(np.array(targets, np.float32), (128, 1)),
    }


def _install_ntff_hook_shim(so_path="/opt/axon/libaxon_pjrt.so"):
    """Provide antenv.axon_hooks (missing in this image) so
    run_bass_kernel_spmd(trace=True) can capture NTFF profiles."""
    import sys as _sys
    import types, ctypes, contextlib
    if "antenv.axon_hooks" in _sys.modules:
        return
    lib = ctypes.CDLL(so_path)
    if not hasattr(lib, "axon_start_nrt_profile"):
        return
    lib.axon_start_nrt_profile.argtypes = [
        ctypes.POINTER(ctypes.c_int64), ctypes.c_size_t]
    lib.axon_start_nrt_profile.restype = ctypes.c_int64
    lib.axon_stop_nrt_profile.argtypes = [ctypes.c_char_p]
    lib.axon_stop_nrt_profile.restype = ctypes.c_int64

    @contextlib.contextmanager
    def _hook(output_dir, device_ids):
        import jax
        jax.devices()
        if device_ids:
            ids = (ctypes.c_int64 * len(device_ids))(*device_ids)
            rc = lib.axon_start_nrt_profile(ids, len(device_ids))
        else:
            rc = lib.axon_start_nrt_profile(None, 0)
        if rc != 0:
            raise RuntimeError(f"axon_start_nrt_profile rc={rc}")
        try:
            yield
        finally:
            n = lib.axon_stop_nrt_profile(str(output_dir).encode())
            print(f"ntff profile: {n} file(s) -> {output_dir}")

    mod = types.ModuleType("antenv.axon_hooks")
    mod.get_axon_ntff_profile_hook = lambda: _hook
    mod.set_axon_ntff_profile_hook = lambda h: None
    import antenv
    antenv.axon_hooks = mod
    _sys.modules["antenv.axon_hooks"] = mod


def kernel(x, cw1, cw2, cw3, threshold_param, scale_weights,
           reconstruction_weight):
    host = _host_scalars(cw1, cw2, cw3, threshold_param, scale_weights,
                         reconstruction_weight)
    nc = build_nc(host)
    in_maps = [
        {"x": np.ascontiguousarray(x[c]), "tgt": host["targets"]}
        for c in range(NCORES)
    ]
    import os
    trace = bool(int(os.environ.get("KERNEL_TRACE", "0")))
    if trace:
        _install_ntff_hook_shim()
    res = run_bass_kernel_spmd(
        nc, in_maps, core_ids=list(range(NCORES)), trace=trace
    )
    if trace:
        print("HW exec time:", res.exec_time_ns, "ns")
        kernel.last_result = res
    out = np.stack([res.results[c]["out"] for c in range(NCORES)], axis=0)
    return out.astype(np.float32)


if __name__ == "__main__":
    host = _host_scalars(
        np.zeros((1, 1, C), np.float32), np.zeros((1, 1, C), np.float32),
        np.zeros((1, 1, C), np.float32), np.array([0.25], np.float32),
        np.zeros((4,), np.float32), np.zeros((1,), np.float32),
    )
    nc = build_nc(host)
    print("built ok")
